# revision 19
# baseline (speedup 1.0000x reference)
"""Trainium2 Bass kernel for nn_Attention_36644660969693.

Multi-head attention block: x[8,32,32,768] -> qkv -> 12-head attention -> wo.
Sharding: data-parallel over batch, one image (1024 tokens) per NeuronCore;
no collectives. ~180 us per core on HW, rel err ~5e-4 vs the fp32 reference.

Per-core design (T=1024 tokens, C=768, 12 heads, hd=64), all matmuls fp16
with fp32 PSUM accumulation:
  - x is loaded in 8 chunks of 128 tokens (gpsimd casting SWDGE) so PE
    transposes start right after the first chunk lands; v weight columns
    stream in parallel on the sync HWDGE ring as raw f32 and are cast to
    fp16 by the then-idle ScalarE. gpsimd DMA priority: x chunks 0-3,
    pair-0 q/k columns, x chunks 4-7, bulk q/k, w_o
  - prep emits only the ch0 halves of the pair-0 q/k tiles before the
    attention loop; the remaining transposes + ch1 q/k halves run as
    fillers inside pair-0's first half-pass, whose AV lag is deepened to 4
    key tiles so the PE never FIFO-stalls on the input stream
  - xT[c,t] via PE transpose of x (fp16)
  - qkT[f,t] = w_qkv tile-stationary @ xT; head h lands at partition
    (h*64)%128 of f-tile h//2, so a head PAIR occupies the two partition
    halves of one tile
  - v[t,f] natural orientation, stored per pair as [v_even | ones | v_odd]
    (192 cols): the AV stationary for the even head is [v|1], for the odd
    head [1|v], so each AV matmul emits the softmax row-sums in the
    complementary 64 output partitions for free; v is computed just-in-time
    inside pair 0's first half-pass, lagged behind the weight-column DMAs
  - scoresT[j,i] per head = kT-tile-stationary @ qT (K=64): the two packed
    heads write the two banks of ONE psum tile back-to-back (disjoint PE
    row groups -> they execute concurrently at ~1.7x), and a single ScalarE
    exp per (key-tile, i-chunk) reads the pair straight from PSUM with the
    1/8 scale fused, writing fp16
  - each pair runs as two i-chunk half-passes with 2-key-tile batched
    score groups; AV accumulation lags exp by two key tiles, and each
    half-pass's final AV steps + normalize are deferred into the next
    half-pass so ScalarE never stalls at boundaries; next-pair qkT tiles
    and (for the last pair) the first half of the output projection
    interleave as fine-grained PE filler (2-6 matmuls per slot)
  - normalize handles both packed heads together: two fast-reciprocals,
    then both 64-partition swap DMAs in flight, then both multiplies, so
    the swap latency is paid once per half-pass
  - out[t,:] = aT-tile-stationary @ w_o, natural layout; output DMAs
    alternate between the sync and scalar HWDGE rings so the final four
    stores drain two at a time
Measured on HW (8 cores, spmd): PE is ~97% busy from first transpose to
last wo matmul at 1 col/cycle streaming (2.0-2.4 GHz DVFS-dependent);
~14 us fixed startup (runtime preamble + DMA-ring latency) and ~9 us
fixed epilogue (per-engine semaphore resets) bracket the compute. fp8
DoubleRow, wider moving operands (>512), DVE/Pool exp offload (incl. a
last-two-key-tile Schraudolph variant) and XBAR DMA transposes were all
measured on HW and rejected (2x-but-accuracy-fatal / ISA-illegal /
net-slower respectively).
"""

import numpy as np

import concourse.bass as bass
import concourse.tile as tile
from concourse import bacc, mybir
from concourse import bass_utils
from concourse import masks

P = 128          # partitions
T = 1024         # tokens per image
C = 768          # model dim
NT = T // P      # 8 token tiles
NC = C // P      # 6 channel tiles
NH = 12          # heads
HD = 64          # head dim
NPAIR = NH // 2  # 6 head pairs
VPW = 192        # v_pad pair block width: [v_even(64) | ones(64) | v_odd(64)]
SCALE = HD ** -0.5
F32 = mybir.dt.float32
F16 = mybir.dt.float16
EXP = mybir.ActivationFunctionType.Exp


def attention_kernel(tc, out_d, x_d, wq_d, wo_d):
    nc = tc.nc
    from contextlib import ExitStack

    with ExitStack() as ctx:
        const_pool = ctx.enter_context(tc.tile_pool(name="const", bufs=1))
        persist = ctx.enter_context(tc.tile_pool(name="persist", bufs=1))
        opool = ctx.enter_context(tc.tile_pool(name="ot", bufs=4))

        identh = const_pool.tile([P, P], F16, tag="identh")
        masks.make_identity(nc, identh[:])

        xT = persist.tile([P, NC * T], F16, tag="xT")        # [c, t] blocks
        wq = persist.tile([P, NC * 2304], F16, tag="wq")     # [c, f] blocks
        qkT = persist.tile([P, 12 * T], F16, tag="qkT")      # [f, t] blocks
        vpad = persist.tile([P, NT * NPAIR * VPW], F16, tag="vpad")
        aT = persist.tile([P, NC * T], F16, tag="aT")        # [c, t] blocks
        wo_sb = persist.tile([P, NC * C], F16, tag="wo")     # [c, c'] blocks

        # ones blocks of v_pad: cols 64:128 of each 192-col pair block
        ones_ap = vpad[:].rearrange(
            "p (blk w) -> p blk w", w=VPW
        )[:, :, HD: 2 * HD]
        nc.vector.memset(ones_ap, 1.0)

        # ---- input DMAs: gpsimd SWDGE casts f32->f16 in flight ----
        # priority order: x, pair-0 q/k columns, v columns, bulk q/k, w_o
        qkcol = [[768, 2], [1, 128]]     # cols 0:128 and 768:896
        bulkcol = [[768, 2], [1, 640]]   # cols 128:768 and 896:1536

        def wq_col_dma(ct, pattern, off):
            dst = wq[:, ct * 2304 + off: (ct + 1) * 2304]
            dst = bass.AP(
                tensor=dst.tensor, offset=dst.offset, ap=[dst.ap[0]] + pattern)
            s = wq_d[ct * P:(ct + 1) * P, off:2304]
            src = bass.AP(
                tensor=s.tensor, offset=s.offset, ap=[s.ap[0]] + pattern)
            nc.gpsimd.dma_start(dst, src)

        # attention-phase pools (also used by the chunked prep below)
        epool = ctx.enter_context(tc.tile_pool(name="E", bufs=2))
        rpool = ctx.enter_context(tc.tile_pool(name="recip", bufs=2))
        pp_s = ctx.enter_context(tc.tile_pool(name="pps", bufs=2, space="PSUM"))
        pp_av = ctx.enter_context(tc.tile_pool(name="ppav", bufs=4, space="PSUM"))

        # ---- chunked x load: 8 DMAs of 128 tokens each; transposes start
        # after the first chunk instead of after half the image ----
        stage_x = ctx.enter_context(tc.tile_pool(name="sx", bufs=8))
        wvpool = ctx.enter_context(tc.tile_pool(name="wvf", bufs=1))
        xcs = []

        def x_chunk_dma(tt):
            xc = stage_x.tile([P, C], F16, tag="xh", name=f"xc{tt}")
            nc.gpsimd.dma_start(xc[:], x_d[tt * P:(tt + 1) * P, :])
            xcs.append(xc)

        def transpose_chunk(tt):
            xc = xcs[tt]
            ps = pp_av.tile([P, 1024], F16, tag="av", name=f"tr{tt}")
            for ct in range(NC):
                nc.tensor.transpose(
                    ps[:, ct * P:(ct + 1) * P],
                    xc[:, ct * P:(ct + 1) * P],
                    identh[:],
                )
            dst = xT[:].rearrange("p (ct t) -> p ct t", ct=NC)[
                :, :, tt * P:(tt + 1) * P]
            src = ps[:, 0:NC * P].rearrange("p (ct k) -> p ct k", ct=NC)
            nc.vector.tensor_copy(dst, src)

        # DMA priority. gpsimd (casting) queue: x chunks 0-3, pair-0 q/k
        # cols, x chunks 4-7, bulk q/k, w_o. sync queue (parallel ring):
        # v columns as raw f32, cast to wq's v region on the idle ScalarE.
        def wv_dma(half):
            wvf = wvpool.tile([P, 3 * 768], F32, tag="wvf", name=f"wvf{half}")
            nc.sync.dma_start(
                wvf[:].rearrange("p (ct f) -> p ct f", ct=3),
                wq_d[half * 384:(half + 1) * 384, 1536:2304].rearrange(
                    "(ct p) f -> p ct f", p=P),
            )
            return wvf
        for tt in range(4):
            x_chunk_dma(tt)
        for ct in range(NC):
            wq_col_dma(ct, qkcol, 0)          # pair-0 q/k columns
        for tt in range(4, NT):
            x_chunk_dma(tt)
        for ct in range(NC):
            wq_col_dma(ct, bulkcol, 128)      # remaining q/k columns
        for ct in range(NC):
            nc.gpsimd.dma_start(wo_sb[:, ct * C:(ct + 1) * C],
                                wo_d[ct * P:(ct + 1) * P, :])

        def cast_wv(half, wvf):
            dst = wq[:].rearrange("p (ct f) -> p ct f", f=2304)[
                :, half * 3:half * 3 + 3, 1536:2304]
            src = wvf[:].rearrange("p (ct f) -> p ct f", ct=3)
            nc.scalar.copy(dst, src)

        qk_psum = {"pool": pp_av, "tag": "av"}

        def emit_qk_half(ft, ch):
            """qkT f-tile ft for token half ch: 6 MMs + one cast."""
            ps = pp_av.tile([P, 512], F32, tag="av", name=f"qkh{ft}{ch}")
            for ct in range(NC):
                nc.tensor.matmul(
                    ps[:],
                    wq[:, ct * 2304 + ft * P: ct * 2304 + ft * P + P],
                    xT[:, ct * T + ch * 512: ct * T + ch * 512 + 512],
                    start=(ct == 0),
                    stop=(ct == NC - 1),
                )
            nc.vector.tensor_copy(
                qkT[:, ft * T + ch * 512: ft * T + ch * 512 + 512], ps[:])

        wvf0 = wv_dma(0)
        for tt in range(4):
            transpose_chunk(tt)
        cast_wv(0, wvf0)
        wvf1 = wv_dma(1)
        cast_wv(1, wvf1)
        emit_qk_half(0, 0)
        emit_qk_half(6, 0)
        # chunks 4-7 + ch1 q/k tiles are emitted as pair-0 ch0 fillers;
        # consumed one GROUP per slot at jt=0..3 of the ch0 half-pass
        hp0_fillers = [
            [lambda: transpose_chunk(4), lambda: transpose_chunk(5)],
            [lambda: transpose_chunk(6), lambda: transpose_chunk(7)],
            [lambda: emit_qk_half(6, 1)],
            [lambda: emit_qk_half(0, 1)],
        ]

        def make_qk_emitter(ft):
            """Returns step(n): emits n accumulation matmuls of the qkT
            f-tile computation, so the work interleaves finely with the
            score stream instead of blocking it."""
            pool, tag = qk_psum["pool"], qk_psum["tag"]
            st = {"ch": 0, "ct": 0, "ps": None}

            def step(n):
                for _ in range(n):
                    ch, ct = st["ch"], st["ct"]
                    if ch >= 2:
                        return
                    if ct == 0:
                        st["ps"] = pool.tile([P, 512], F32, tag=tag, name="ps_qk")
                    nc.tensor.matmul(
                        st["ps"][:],
                        wq[:, ct * 2304 + ft * P: ct * 2304 + ft * P + P],
                        xT[:, ct * T + ch * 512: ct * T + ch * 512 + 512],
                        start=(ct == 0),
                        stop=(ct == NC - 1),
                    )
                    if ct == NC - 1:
                        nc.vector.tensor_copy(
                            qkT[:, ft * T + ch * 512: ft * T + ch * 512 + 512],
                            st["ps"][:])
                        st["ch"], st["ct"] = ch + 1, 0
                    else:
                        st["ct"] = ct + 1
            return step

        # ---- attention, one head pair at a time ----
        wo_state = {}

        def emit_wo_half(tt, half):
            """Half of one output t-tile projection (6 matmuls)."""
            if half == 0:
                po1 = pp_av.tile([P, 512], F32, tag="av", name="po1")
                wo_state[tt] = po1
                for ct in range(NC):
                    lhsT = aT[:, ct * T + tt * P: ct * T + tt * P + P]
                    nc.tensor.matmul(po1[:], lhsT, wo_sb[:, ct * C: ct * C + 512],
                                     start=(ct == 0), stop=(ct == NC - 1))
            else:
                po1 = wo_state.pop(tt)
                po2 = pp_av.tile([P, 512], F32, tag="av", name="po2")
                for ct in range(NC):
                    lhsT = aT[:, ct * T + tt * P: ct * T + tt * P + P]
                    nc.tensor.matmul(po2[:, :256], lhsT,
                                     wo_sb[:, ct * C + 512: ct * C + C],
                                     start=(ct == 0), stop=(ct == NC - 1))
                ot = opool.tile([P, C], F32, tag="ot", name="ot")
                nc.scalar.copy(ot[:, 0:512], po1[:])
                nc.vector.tensor_copy(ot[:, 512:C], po2[:, :256])
                # alternate output DMAs across the two HWDGE rings so the
                # final four stores don't serialize on one queue
                eng = nc.sync if tt % 2 == 0 else nc.scalar
                eng.dma_start(out_d[tt * P:(tt + 1) * P, :], ot[:])

        def emit_wo(tts):
            for tt in tts:
                emit_wo_half(tt, 0)
                emit_wo_half(tt, 1)

        def vslice(jt, hp, h):
            """[128,128] AV stationary: even head [v|1], odd head [1|v]."""
            base = jt * NPAIR * VPW + hp * VPW + (0 if h == 0 else HD)
            return vpad[:, base: base + P]

        def emit_v_chunk(tt, chunk):
            """One 6-matmul chunk of v[t-tile tt] (chunk 0: heads 0-7,
            chunk 1: heads 8-11)."""
            for (foff, fw) in ((0, 512), (512, 256))[chunk:chunk + 1]:
                ps = pp_av.tile([P, 512], F32, tag="av", name="ps_v")
                for ct in range(NC):
                    nc.tensor.matmul(
                        ps[:, :fw],
                        xT[:, ct * T + tt * P: ct * T + tt * P + P],
                        wq[:, ct * 2304 + 1536 + foff: ct * 2304 + 1536 + foff + fw],
                        start=(ct == 0),
                        stop=(ct == NC - 1),
                    )
                npr = fw // 128
                src = ps[:, :fw].rearrange("p (m two d) -> p m two d", two=2, d=HD)
                base = tt * NPAIR * VPW + (foff // 128) * VPW
                dst = vpad[:, base: base + npr * VPW].rearrange(
                    "p (m blk) -> p m blk", blk=VPW
                )
                nc.vector.tensor_copy(dst[:, :, 0:HD], src[:, :, 0, :])
                nc.vector.tensor_copy(dst[:, :, 2 * HD:VPW], src[:, :, 1, :])

        def normalize_pair(a0, a1, hp, ch):
            # both heads interleaved: recips first, then both swap DMAs in
            # flight together, then the multiplies (hides DMA+sem latency)
            r0 = rpool.tile([P, 512], F32, tag="r", name=f"r0{ch}")
            r1 = rpool.tile([P, 512], F32, tag="r", name=f"r1{ch}")
            r2a = rpool.tile([P, 512], F32, tag="r2", name=f"r2a{ch}")
            r2b = rpool.tile([P, 512], F32, tag="r2", name=f"r2b{ch}")
            dst = aT[:, hp * T + ch * 512: hp * T + ch * 512 + 512]
            # full-partition approx reciprocal (custom DVE op needs base
            # partition 0); the non-rowsum half of r is garbage, never read
            nc.vector.reciprocal_approx_fast(r0[:, :], a0[:, :])
            nc.vector.reciprocal_approx_fast(r1[:, :], a1[:, :])
            nc.sync.dma_start(r2a[0:HD, :], r0[HD:P, :])
            nc.sync.dma_start(r2b[HD:P, :], r1[0:HD, :])
            nc.vector.tensor_mul(dst[0:HD, :], a0[0:HD, :], r2a[0:HD, :])
            nc.vector.tensor_mul(dst[HD:P, :], a1[HD:P, :], r2b[HD:P, :])

        def eoff(jt, ch, h):
            return jt * 2048 + ch * T + h * 512

        def emit_scores_exp(hp, jt, ch, E):
            """Both packed heads' scores for one i-chunk into ONE 2-bank
            psum tile (forces the row-group pair to issue back-to-back),
            then a single exp over the pair."""
            qblk = hp * T
            kblk = (6 + hp) * T
            s = pp_s.tile([P, T], F32, tag="s", name="s")
            nc.tensor.matmul(
                s[:, 0:512],
                qkT[0:HD, kblk + jt * P: kblk + jt * P + P],
                qkT[0:HD, qblk + ch * 512: qblk + ch * 512 + 512],
                start=True, stop=True,
            )
            nc.tensor.matmul(
                s[:, 512:1024],
                qkT[HD:P, kblk + jt * P: kblk + jt * P + P],
                qkT[HD:P, qblk + ch * 512: qblk + ch * 512 + 512],
                start=True, stop=True,
            )
            nc.scalar.activation(E[:, eoff(jt, ch, 0): eoff(jt, ch, 0) + T],
                                 s[:], EXP, scale=SCALE)

        pending_tail = None   # previous half-pass: final AV steps + normalizes

        for hp in range(NPAIR):
            E = epool.tile([P, NT * 2048], F16, tag="E", name="E")
            last = hp == NPAIR - 1

            for ch in range(2):
                # AV accumulators are allocated lazily at the first av_step:
                # by then the psum ring has advanced onto long-drained filler
                # slots, so the allocation doesn't chain the new half-pass
                # onto the previous accumulators' (still-running) normalize
                acc = []

                def get_acc(ch=ch, acc=acc):
                    if not acc:
                        acc.append(pp_av.tile([P, 512], F32, tag="av",
                                              name=f"a0c{ch}"))
                        acc.append(pp_av.tile([P, 512], F32, tag="av",
                                              name=f"a1c{ch}"))
                    return acc

                def av_step(jt, ch=ch, get_acc=get_acc, hp=hp, E=E):
                    a0, a1 = get_acc()
                    for a, h in ((a0, 0), (a1, 1)):
                        nc.tensor.matmul(
                            a[:],
                            vslice(jt, hp, h),
                            E[:, eoff(jt, ch, h): eoff(jt, ch, h) + 512],
                            start=(jt == 0),
                            stop=(jt == NT - 1),
                        )

                # fine-grained filler schedule for this half-pass
                if hp == 0 and ch == 1:
                    qk_steps = [make_qk_emitter(1), make_qk_emitter(7)]
                elif 0 < hp < NPAIR - 1:
                    qk_steps = [make_qk_emitter(hp + 1 if ch == 0 else 6 + hp + 1)]
                else:
                    qk_steps = []

                # pair-0 ch0 runs while x chunks 4-7 / v columns stream in:
                # v chunks are emitted lagged and AV lags 4 j-tiles so the
                # PE never FIFO-stalls on the input DMAs.
                first = hp == 0 and ch == 0
                av_lag = 4 if first else 2

                for jtp in range(0, NT, 2):
                    # two adjacent score-pair groups: their stationaries sit
                    # in disjoint PE row groups, so weight loads pre-overlap
                    for jt in (jtp, jtp + 1):
                        emit_scores_exp(hp, jt, ch, E)
                        if jt == 0 and pending_tail is not None:
                            pending_tail()
                            pending_tail = None
                    for jt in (jtp, jtp + 1):
                        if jt >= av_lag:
                            av_step(jt - av_lag)
                        for q in qk_steps:
                            q(2)
                        if first and hp0_fillers:
                            for f in hp0_fillers.pop(0):
                                f()
                        if first:
                            if jt >= 1:
                                emit_v_chunk(jt - 1, 0)
                            if jt >= 2:
                                emit_v_chunk(jt - 2, 1)
                        if last and ch == 1 and jt >= 2:
                            # shifted one slot later than the normalize that
                            # produces pair-5 ch0's aT, so po1 doesn't stall
                            emit_wo_half((jt - 2) // 2, (jt - 2) % 2)
                if first:
                    emit_v_chunk(NT - 1, 0)
                    emit_v_chunk(NT - 2, 1)
                    emit_v_chunk(NT - 1, 1)
                for q in qk_steps:
                    q(2 * NC)   # drain any remainder

                def make_tail(av_step=av_step, get_acc=get_acc, hp=hp,
                              ch=ch, av_lag=av_lag):
                    def run():
                        for j in range(NT - av_lag, NT):
                            av_step(j)
                        a0, a1 = get_acc()
                        normalize_pair(a0, a1, hp, ch)
                    return run

                if last and ch == 1:
                    # no next half-pass to defer into: finish AV immediately
                    # (lag already satisfied for jts < NT - av_lag)
                    for j in range(NT - av_lag, NT):
                        av_step(j)
                    a0, a1 = get_acc()
                    normalize_pair(a0, a1, hp, ch)
                    pending_tail = None
                else:
                    pending_tail = make_tail()

        emit_wo_half(3, 0)
        emit_wo_half(3, 1)
        emit_wo(range(NT // 2, NT))


_CACHED = {}
def build_program():
    if "nc" in _CACHED:
        return _CACHED["nc"]
    nc = bacc.Bacc("TRN2", target_bir_lowering=False, debug=False, num_devices=8)
    x_d = nc.dram_tensor("x", [T, C], F32, kind="ExternalInput").ap()
    wq_d = nc.dram_tensor("w_qkv", [C, 3 * C], F32, kind="ExternalInput").ap()
    wo_d = nc.dram_tensor("w_o", [C, C], F32, kind="ExternalInput").ap()
    out_d = nc.dram_tensor("out", [T, C], F32, kind="ExternalOutput").ap()
    with tile.TileContext(nc) as tc:
        attention_kernel(tc, out_d, x_d, wq_d, wo_d)
    nc.compile()
    _CACHED["nc"] = nc
    return nc


def kernel(x, w_qkv, w_o, _trace=False, _trace_cores=None):
    nc = build_program()
    x = np.ascontiguousarray(np.asarray(x, dtype=np.float32))
    w_qkv = np.ascontiguousarray(np.asarray(w_qkv, dtype=np.float32))
    w_o = np.ascontiguousarray(np.asarray(w_o, dtype=np.float32))
    bs = x.shape[0]
    in_maps = [
        {"x": x[b].reshape(T, C), "w_qkv": w_qkv, "w_o": w_o} for b in range(bs)
    ]
    res = bass_utils.run_bass_kernel_spmd(
        nc, in_maps, core_ids=list(range(bs)), trace=_trace,
        trace_cores=_trace_cores,
    )
    out = np.stack([res.results[b]["out"].reshape(32, 32, C) for b in range(bs)])
    if _trace:
        return out, res
    return out

